# revision 11
# baseline (speedup 1.0000x reference)
"""APPNP forward on 8 Trainium2 NeuronCores.

Reference computation:
    h = features; 10x: h = 0.9 * (segment_sum(h*ns)[src->dst] * nd) + 0.1 * h0

Distribution: nodes sharded across 8 cores (12544 padded rows each); edges
partitioned by destination core.  Each step:
  1. AllGather the scaled feature table g = h*norm_src (bf16) to every core.
  2. Bulk-gather per-edge source rows with InstDMAGatherAnt (256B descriptors,
     4 table views to keep indices within int16: view j covers src%4==j at
     row stride 1024B, so the gathered 256B row always starts at node src).
  3. Segment-sum by dst window (128 wide) via one-hot matmuls into PSUM;
     one-hot built on DVE with a broadcast is_equal against an iota row.
  4. Blend with precomputed per-node coefficients; write next g shard.

All data-dependent structure (edge ordering, padding, schedule) is computed
on host; the device program is identical across cores (SPMD).
"""

import sys

sys.path.insert(0, "/opt/trn_rl_repo")

import numpy as np
import ml_dtypes

BF16 = ml_dtypes.bfloat16

K_LAYERS = 10
ALPHA = 0.1
N_NODES = 100_000
D_FEAT = 64
M_CORES = 8

W = 128   # dst window width (psum partitions)
CH = 128  # edges per matmul chunk (contraction dim)
V = 4     # table views (src % 4)


def _preprocess(src, dst, n_nodes, M, G):
    """Host-side schedule. Edge order: (core | superchunk | view | window),
    each (window, view) run padded to a multiple of 128 edges with sentinels
    (srcidx 0, dstw -1). Chunk counts are shared across cores (max over
    cores) so all cores run the same program."""
    E = src.shape[0]
    shard = -(-n_nodes // M)
    shard = -(-shard // W) * W
    npad = shard * M
    nw = shard // W                      # windows per core
    nsc = -(-nw // G)                    # superchunks per core

    core = dst // shard
    ldst = dst - core * shard
    w = ldst // W
    dw = (ldst - w * W).astype(np.int32)
    v = (src % V).astype(np.int64)
    sc = w // G

    # counts per (core, window, view) -> shared chunk counts
    flat = (core * nw + w) * V + v
    cnt = np.bincount(flat, minlength=M * nw * V).reshape(M, nw, V)
    K_wv = -(-cnt.max(axis=0) // CH)     # [nw, V]
    # every window needs >= 1 chunk so the blend runs
    empty = K_wv.sum(axis=1) == 0
    K_wv[empty, 0] = 1

    # schedule order of (w, v) blocks: for sc, for v, for w in sc
    blocks = []  # (w, v) in edge order
    for s in range(nsc):
        ws = range(s * G, min((s + 1) * G, nw))
        for vv in range(V):
            for ww in ws:
                if K_wv[ww, vv] > 0:
                    blocks.append((ww, vv))
    bsizes = np.array([K_wv[ww, vv] * CH for ww, vv in blocks], dtype=np.int64)
    bstarts = np.concatenate([[0], np.cumsum(bsizes)])
    e_pad = int(bstarts[-1])
    nchunk = e_pad // CH
    # block id lookup: (w, v) -> block index
    bid = np.full((nw, V), -1, dtype=np.int64)
    for i, (ww, vv) in enumerate(blocks):
        bid[ww, vv] = i

    # place edges
    key = (core * nsc + sc) * (V * nw) + v * nw + w
    order = np.argsort(key, kind="stable")
    key_s = key[order]
    cnt_flat = np.bincount(key, minlength=M * nsc * V * nw)
    starts = np.concatenate([[0], np.cumsum(cnt_flat)])
    rank = np.arange(E) - starts[key_s]
    c_s = core[order]
    pos = bstarts[bid[w[order], v[order]]] + rank

    srcidx_lin = np.zeros((M, e_pad), dtype=np.int64)   # table row = src >> 2
    dstw_lin = np.full((M, e_pad), -1.0, dtype=np.float32)
    srcidx_lin[c_s, pos] = src[order] >> 2
    dstw_lin[c_s, pos] = dw[order]

    dstw_t = np.ascontiguousarray(
        dstw_lin.reshape(M, nchunk, CH).transpose(0, 2, 1)
    ).astype(BF16)

    # gather calls: one per (sc, v), chunk range [k0, k1)
    calls = []  # (sc, v, k0, k1)
    chunk_of_block = np.concatenate([[0], np.cumsum(bsizes // CH)])
    i = 0
    for s in range(nsc):
        ws = range(s * G, min((s + 1) * G, nw))
        for vv in range(V):
            nb = sum(1 for ww in ws if K_wv[ww, vv] > 0)
            if nb == 0:
                continue
            k0, k1 = chunk_of_block[i], chunk_of_block[i + nb]
            calls.append((s, vv, int(k0), int(k1)))
            i += nb
    assert i == len(blocks)

    # int16 idx array in per-call 16-wrapped layout, replicated to 128 parts
    idx16 = np.zeros((M, 128, nchunk * CH // 16), dtype=np.int16)
    for (_s, _v, k0, k1) in calls:
        seg = srcidx_lin[:, k0 * CH:k1 * CH]             # [M, n]
        wrapped = seg.reshape(M, -1, 16).transpose(0, 2, 1)  # [M, 16, n/16]
        idx16[:, :, k0 * CH // 16:k1 * CH // 16] = np.tile(wrapped, (1, 8, 1))

    # chunk -> (window, start?, stop?) schedule
    wmap = np.empty(nchunk, dtype=np.int64)
    for i, (ww, vv) in enumerate(blocks):
        wmap[chunk_of_block[i]:chunk_of_block[i + 1]] = ww
    tot = np.bincount(wmap, minlength=nw)
    seen = np.zeros(nw, dtype=np.int64)
    startf = np.zeros(nchunk, dtype=bool)
    stopf = np.zeros(nchunk, dtype=bool)
    for k in range(nchunk):
        ww = wmap[k]
        startf[k] = seen[ww] == 0
        seen[ww] += 1
        stopf[k] = seen[ww] == tot[ww]

    # superchunk chunk ranges
    sc_ranges = []
    for s in range(nsc):
        ks = [c for c in calls if c[0] == s]
        sc_ranges.append((min(k0 for _, _, k0, _ in ks), max(k1 for _, _, _, k1 in ks)))

    return dict(
        shard=shard, npad=npad, nw=nw, nsc=nsc, G=G, nchunk=nchunk,
        K_wv=K_wv, calls=calls, sc_ranges=sc_ranges,
        wmap=wmap, startf=startf, stopf=stopf,
        srcidx=idx16, dstw_t=dstw_t, e_pad=e_pad,
    )


def _build_nc(meta, M, D, steps):
    from concourse import bass, bacc, tile, mybir

    dt = mybir.dt
    shard, npad, nw = meta["shard"], meta["npad"], meta["nw"]
    nchunk, G = meta["nchunk"], meta["G"]
    calls, sc_ranges = meta["calls"], meta["sc_ranges"]
    wmap, startf, stopf = meta["wmap"], meta["startf"], meta["stopf"]
    max_call = max(k1 - k0 for _, _, k0, k1 in calls)
    max_sc = max(k1 - k0 for k0, k1 in sc_ranges)

    nc = bacc.Bacc("TRN2", target_bir_lowering=False, debug=False, num_devices=M)

    g0 = nc.dram_tensor("g0", [shard, D], dt.bfloat16, kind="ExternalInput").ap()
    srcidx = nc.dram_tensor("srcidx", [128, nchunk * CH // 16], dt.int16,
                            kind="ExternalInput").ap()
    dstw = nc.dram_tensor("dstw", [CH, nchunk], dt.bfloat16, kind="ExternalInput").ap()
    acoef = nc.dram_tensor("acoef", [W, nw], dt.float32, kind="ExternalInput").ap()
    bcoef = nc.dram_tensor("bcoef", [W, nw * D], dt.bfloat16, kind="ExternalInput").ap()
    acoef2 = nc.dram_tensor("acoef2", [W, nw], dt.float32, kind="ExternalInput").ap()
    bcoef2 = nc.dram_tensor("bcoef2", [W, nw * D], dt.float32, kind="ExternalInput").ap()
    iota_in = nc.dram_tensor("iota", [CH, W], dt.bfloat16, kind="ExternalInput").ap()
    out = nc.dram_tensor("out", [shard, D], dt.float32, kind="ExternalOutput").ap()

    with tile.TileContext(nc) as tc:
        with (
            tc.tile_pool(name="dram", bufs=1, space="DRAM") as dram,
            tc.tile_pool(name="const", bufs=1) as const,
            tc.tile_pool(name="iop", bufs=6) as iop,
            tc.tile_pool(name="msgp", bufs=5) as msgp,
            tc.tile_pool(name="ohp", bufs=4) as ohp,
            tc.tile_pool(name="psp", bufs=8, space="PSUM") as psp,
            tc.tile_pool(name="resp", bufs=3) as resp,
        ):
            T_ins = [
                dram.tile([shard, D], dt.bfloat16, tag=f"ti{s}", name=f"T_in{s}")
                for s in range(steps)
            ]
            T_outs = [
                dram.tile([npad + 4, D], dt.bfloat16, addr_space="Shared",
                          tag=f"to{s}", name=f"T_out{s}")
                for s in range(steps)
            ]

            a_t = const.tile_from(acoef)
            b_t = const.tile_from(bcoef)
            a2_t = const.tile_from(acoef2)
            b2_t = const.tile_from(bcoef2)
            io_t = const.tile_from(iota_in)

            nc.sync.dma_start(out=T_ins[0][:, :], in_=g0[:, :])

            for step in range(steps):
                last = step == steps - 1
                T_in = T_ins[step]
                T_out = T_outs[step]
                T_in_next = T_ins[step + 1] if not last else None
                nc.gpsimd.collective_compute(
                    "AllGather",
                    mybir.AluOpType.bypass,
                    replica_groups=[list(range(M))],
                    ins=[T_in.opt()],
                    outs=[T_out[:npad, :].opt()],
                )
                # 4-node-stride views of the table: view j covers src%4 == j.
                # Row r of view j = elements [64j+256r, 64j+256r+128) = the
                # 256B row starting at node 4r+j.
                flat = T_out[:, :].rearrange("n d -> (n d)")
                nr = npad // 4
                views = []
                for j in range(V):
                    vj = flat[64 * j:64 * j + nr * 256]
                    views.append(vj.rearrange("(r s) -> r s", s=256)[:, 0:128])

                ci = 0  # call index
                for s, (ks0, ks1) in enumerate(sc_ranges):
                    # gather + one-hot per call
                    sc_msgs = []  # (k0, k1, msg tile, oh tile)
                    while ci < len(calls) and calls[ci][0] == s:
                        _, vv, k0, k1 = calls[ci]
                        ncall = k1 - k0
                        idx_t = iop.tile([128, max_call * CH // 16], dt.int16,
                                         tag="idx")
                        dw_t = iop.tile([CH, max_call], dt.bfloat16, tag="dw")
                        msg_t = msgp.tile([CH, max_call, 128], dt.bfloat16,
                                          tag="msg")
                        oh_t = ohp.tile([CH, max_call, W], dt.bfloat16, tag="oh")
                        nc.sync.dma_start(
                            out=idx_t[:, :ncall * CH // 16],
                            in_=srcidx[:, k0 * CH // 16:k1 * CH // 16],
                        )
                        nc.sync.dma_start(out=dw_t[:, :ncall], in_=dstw[:, k0:k1])
                        nc.gpsimd.dma_gather(
                            out_ap=msg_t[:, :ncall, :],
                            in_ap=views[vv],
                            idxs_ap=idx_t[:, :ncall * CH // 16],
                            num_idxs=ncall * CH,
                            num_idxs_reg=ncall * CH,
                            elem_size=128,
                            elem_step=256,
                            single_packet=False,
                        )
                        nc.vector.tensor_tensor(
                            out=oh_t[:, :ncall, :],
                            in0=dw_t[:, :ncall, None].to_broadcast([CH, ncall, W]),
                            in1=io_t[:, None, :].to_broadcast([CH, ncall, W]),
                            op=mybir.AluOpType.is_equal,
                        )
                        sc_msgs.append((k0, k1, msg_t, oh_t))
                        ci += 1
                    # matmul chunks + blend finished windows
                    gw_t = resp.tile([W, G, D], dt.float32, tag="gw")
                    psums = {}
                    for k0, k1, msg_t, oh_t in sc_msgs:
                        for k in range(k0, k1):
                            ww = int(wmap[k])
                            if startf[k]:
                                psums[ww] = psp.tile([W, D], dt.float32, tag="ps", name="ps")
                            nc.tensor.matmul(
                                out=psums[ww][:, :],
                                lhsT=oh_t[:, k - k0, :],
                                rhs=msg_t[:, k - k0, 0:64],
                                start=bool(startf[k]),
                                stop=bool(stopf[k]),
                            )
                            if stopf[k]:
                                wi = ww - s * G
                                ca = (a2_t if last else a_t)[:, ww:ww + 1]
                                cb = (b2_t if last else b_t)[:, ww * D:(ww + 1) * D]
                                tmp_t = resp.tile([W, D], dt.float32, tag="tmp")
                                nc.vector.tensor_scalar(
                                    out=tmp_t[:, :], in0=psums[ww][:, :],
                                    scalar1=ca, scalar2=None,
                                    op0=mybir.AluOpType.mult,
                                )
                                nc.vector.tensor_tensor(
                                    out=gw_t[:, wi, :], in0=tmp_t[:, :], in1=cb,
                                    op=mybir.AluOpType.add,
                                )
                    # store the blended windows of this superchunk
                    w0 = s * G
                    nwin = min(G, nw - w0)
                    tgt = out if last else T_in_next[:, :]
                    if last:
                        o32 = resp.tile([W, G, D], dt.float32, tag="o32")
                        nc.vector.tensor_copy(
                            out=o32[:, :nwin, :], in_=gw_t[:, :nwin, :]
                        )
                        nc.sync.dma_start(
                            out=out[w0 * W:(w0 + nwin) * W, :]
                                .rearrange("(a p) d -> p a d", p=W),
                            in_=o32[:, :nwin, :],
                        )
                    else:
                        g16 = resp.tile([W, G, D], dt.bfloat16, tag="g16")
                        nc.vector.tensor_copy(
                            out=g16[:, :nwin, :], in_=gw_t[:, :nwin, :]
                        )
                        nc.sync.dma_start(
                            out=T_in_next[w0 * W:(w0 + nwin) * W, :]
                                .rearrange("(a p) d -> p a d", p=W),
                            in_=g16[:, :nwin, :],
                        )
    nc.compile()
    return nc


def _make_inputs(features, src, dst, meta, M, D, alpha):
    n = features.shape[0]
    shard, npad, nw = meta["shard"], meta["npad"], meta["nw"]

    deg_out = np.bincount(src, minlength=n).astype(np.float32)
    deg_in = np.bincount(dst, minlength=n).astype(np.float32)
    ns = np.clip(deg_out, 1.0, None) ** -0.5
    nd = np.clip(deg_in, 1.0, None) ** -0.5

    ns_pad = np.ones(npad, dtype=np.float32)
    nd_pad = np.ones(npad, dtype=np.float32)
    h0_pad = np.zeros((npad, D), dtype=np.float32)
    ns_pad[:n] = ns
    nd_pad[:n] = nd
    h0_pad[:n] = features

    iota = np.tile(np.arange(W, dtype=np.float32), (CH, 1)).astype(BF16)

    in_maps = []
    for c in range(M):
        sl = slice(c * shard, (c + 1) * shard)
        a = ((1.0 - alpha) * nd_pad[sl] * ns_pad[sl]).astype(np.float32)
        b = (alpha * h0_pad[sl] * ns_pad[sl][:, None]).astype(np.float32)
        a2 = ((1.0 - alpha) * nd_pad[sl]).astype(np.float32)
        b2 = (alpha * h0_pad[sl]).astype(np.float32)
        g0 = (h0_pad[sl] * ns_pad[sl][:, None]).astype(BF16)

        def wmaj(x):  # [shard, ...] -> [W, nw * ...]
            x = x.reshape(nw, W, -1).transpose(1, 0, 2)
            return np.ascontiguousarray(x.reshape(W, -1))

        in_maps.append({
            "g0": np.ascontiguousarray(g0),
            "srcidx": meta["srcidx"][c],
            "dstw": meta["dstw_t"][c],
            "acoef": wmaj(a).astype(np.float32),
            "bcoef": wmaj(b).astype(BF16),
            "acoef2": wmaj(a2).astype(np.float32),
            "bcoef2": wmaj(b2).astype(np.float32),
            "iota": iota,
        })
    return in_maps


_NC_CACHE = {}


def build_all(features, src, dst, *, n_nodes=None, M=M_CORES, D=D_FEAT,
              steps=K_LAYERS, alpha=ALPHA, G=7):
    n_nodes = n_nodes or features.shape[0]
    src = np.asarray(src).astype(np.int64)
    dst = np.asarray(dst).astype(np.int64)
    meta = _preprocess(src, dst, n_nodes, M, G)
    key = (meta["nchunk"], tuple(meta["K_wv"].ravel()[:16]), steps, M, G)
    if key not in _NC_CACHE:
        _NC_CACHE[key] = _build_nc(meta, M, D, steps)
    nc = _NC_CACHE[key]
    in_maps = _make_inputs(np.asarray(features, np.float32), src, dst, meta, M, D, alpha)
    return nc, in_maps, meta


def kernel(features, src, dst, *, trace=False, **kw):
    from concourse.bass_utils import run_bass_kernel_spmd

    features = np.asarray(features)
    nc, in_maps, meta = build_all(features, src, dst, **kw)
    res = run_bass_kernel_spmd(nc, in_maps, core_ids=list(range(M_CORES)), trace=trace)
    n = features.shape[0]
    h = np.concatenate([res.results[c]["out"] for c in range(M_CORES)], axis=0)
    out = np.ascontiguousarray(h[:n]).astype(np.float32)
    if trace:
        kernel.last_results = res
    return out


# revision 15
# speedup vs baseline: 3.2859x; 3.2859x over previous
"""APPNP forward on 8 Trainium2 NeuronCores.

Reference computation:
    h = features; 10x: h = 0.9 * (segment_sum(h*ns)[src->dst] * nd) + 0.1 * h0

Distribution: nodes sharded across 8 cores (12544 padded rows each); edges
partitioned by destination core.  Each step:
  1. AllGather the scaled feature table g = h*norm_src (bf16) to every core.
  2. Bulk-gather per-edge source rows with InstDMAGatherAnt (256B descriptors,
     4 table views to keep indices within int16: view j covers src%4==j at
     row stride 1024B, so the gathered 256B row always starts at node src).
  3. Segment-sum by dst window (128 wide) via one-hot matmuls into PSUM;
     one-hot built on DVE with a broadcast is_equal against an iota row.
  4. Blend with precomputed per-node coefficients; write next g shard.

All data-dependent structure (edge ordering, padding, schedule) is computed
on host; the device program is identical across cores (SPMD).
"""

import sys

sys.path.insert(0, "/opt/trn_rl_repo")

import numpy as np
import ml_dtypes

BF16 = ml_dtypes.bfloat16

K_LAYERS = 10
ALPHA = 0.1
N_NODES = 100_000
D_FEAT = 64
M_CORES = 8

W = 128   # dst window width (psum partitions)
CH = 128  # edges per matmul chunk (contraction dim)
V = 4     # table views (src % 4)


def _preprocess(src, dst, n_nodes, M, G):
    """Host-side schedule. Edge order: (core | superchunk | view | window),
    each (window, view) run padded to a multiple of 128 edges with sentinels
    (srcidx 0, dstw -1). Chunk counts are shared across cores (max over
    cores) so all cores run the same program."""
    E = src.shape[0]
    shard = -(-n_nodes // M)
    shard = -(-shard // W) * W
    npad = shard * M
    nw = shard // W                      # windows per core
    nsc = -(-nw // G)                    # superchunks per core

    core = dst // shard
    ldst = dst - core * shard
    w = ldst // W
    dw = (ldst - w * W).astype(np.int32)
    v = (src % V).astype(np.int64)
    sc = w // G

    # counts per (core, window, view) -> shared chunk counts
    flat = (core * nw + w) * V + v
    cnt = np.bincount(flat, minlength=M * nw * V).reshape(M, nw, V)
    K_wv = -(-cnt.max(axis=0) // CH)     # [nw, V]
    # every window needs >= 1 chunk so the blend runs
    empty = K_wv.sum(axis=1) == 0
    K_wv[empty, 0] = 1

    # schedule order of (w, v) blocks: for sc, for v, for w in sc
    blocks = []  # (w, v) in edge order
    for s in range(nsc):
        ws = range(s * G, min((s + 1) * G, nw))
        for vv in range(V):
            for ww in ws:
                if K_wv[ww, vv] > 0:
                    blocks.append((ww, vv))
    bsizes = np.array([K_wv[ww, vv] * CH for ww, vv in blocks], dtype=np.int64)
    bstarts = np.concatenate([[0], np.cumsum(bsizes)])
    e_pad = int(bstarts[-1])
    nchunk = e_pad // CH
    # block id lookup: (w, v) -> block index
    bid = np.full((nw, V), -1, dtype=np.int64)
    for i, (ww, vv) in enumerate(blocks):
        bid[ww, vv] = i

    # place edges
    key = (core * nsc + sc) * (V * nw) + v * nw + w
    order = np.argsort(key, kind="stable")
    key_s = key[order]
    cnt_flat = np.bincount(key, minlength=M * nsc * V * nw)
    starts = np.concatenate([[0], np.cumsum(cnt_flat)])
    rank = np.arange(E) - starts[key_s]
    c_s = core[order]
    pos = bstarts[bid[w[order], v[order]]] + rank

    srcidx_lin = np.zeros((M, e_pad), dtype=np.int64)   # table row = src >> 2
    dstw_lin = np.full((M, e_pad), -1.0, dtype=np.float32)
    srcidx_lin[c_s, pos] = src[order] >> 2
    dstw_lin[c_s, pos] = dw[order]

    dstw_t = np.ascontiguousarray(
        dstw_lin.reshape(M, nchunk, CH).transpose(0, 2, 1)
    ).astype(BF16)

    # gather calls: one per (sc, v), chunk range [k0, k1)
    calls = []  # (sc, v, k0, k1)
    chunk_of_block = np.concatenate([[0], np.cumsum(bsizes // CH)])
    i = 0
    for s in range(nsc):
        ws = range(s * G, min((s + 1) * G, nw))
        for vv in range(V):
            nb = sum(1 for ww in ws if K_wv[ww, vv] > 0)
            if nb == 0:
                continue
            k0, k1 = chunk_of_block[i], chunk_of_block[i + nb]
            calls.append((s, vv, int(k0), int(k1)))
            i += nb
    assert i == len(blocks)

    # int16 idx array in per-call 16-wrapped layout, replicated to 128 parts
    idx16 = np.zeros((M, 128, nchunk * CH // 16), dtype=np.int16)
    for (_s, _v, k0, k1) in calls:
        seg = srcidx_lin[:, k0 * CH:k1 * CH]             # [M, n]
        wrapped = seg.reshape(M, -1, 16).transpose(0, 2, 1)  # [M, 16, n/16]
        idx16[:, :, k0 * CH // 16:k1 * CH // 16] = np.tile(wrapped, (1, 8, 1))

    # chunk -> (window, start?, stop?) schedule
    wmap = np.empty(nchunk, dtype=np.int64)
    for i, (ww, vv) in enumerate(blocks):
        wmap[chunk_of_block[i]:chunk_of_block[i + 1]] = ww
    tot = np.bincount(wmap, minlength=nw)
    seen = np.zeros(nw, dtype=np.int64)
    startf = np.zeros(nchunk, dtype=bool)
    stopf = np.zeros(nchunk, dtype=bool)
    for k in range(nchunk):
        ww = wmap[k]
        startf[k] = seen[ww] == 0
        seen[ww] += 1
        stopf[k] = seen[ww] == tot[ww]

    # superchunk chunk ranges
    sc_ranges = []
    for s in range(nsc):
        ks = [c for c in calls if c[0] == s]
        sc_ranges.append((min(k0 for _, _, k0, _ in ks), max(k1 for _, _, _, k1 in ks)))

    return dict(
        shard=shard, npad=npad, nw=nw, nsc=nsc, G=G, nchunk=nchunk,
        K_wv=K_wv, calls=calls, sc_ranges=sc_ranges,
        wmap=wmap, startf=startf, stopf=stopf,
        srcidx=idx16, dstw_t=dstw_t, e_pad=e_pad,
    )


def _build_nc(meta, M, D, steps, ablate=frozenset(), subc=None, sp=False):
    from concourse import bass, bacc, tile, mybir

    dt = mybir.dt
    shard, npad, nw = meta["shard"], meta["npad"], meta["nw"]
    nchunk, G = meta["nchunk"], meta["G"]
    calls, sc_ranges = meta["calls"], meta["sc_ranges"]
    wmap, startf, stopf = meta["wmap"], meta["startf"], meta["stopf"]
    max_call = max(k1 - k0 for _, _, k0, k1 in calls)
    max_sc = max(k1 - k0 for k0, k1 in sc_ranges)

    nc = bacc.Bacc("TRN2", target_bir_lowering=False, debug=False, num_devices=M)

    g0 = nc.dram_tensor("g0", [shard, D], dt.bfloat16, kind="ExternalInput").ap()
    srcidx = nc.dram_tensor("srcidx", [128, nchunk * CH // 16], dt.int16,
                            kind="ExternalInput").ap()
    dstw = nc.dram_tensor("dstw", [CH, nchunk], dt.bfloat16, kind="ExternalInput").ap()
    acoef = nc.dram_tensor("acoef", [W, nw], dt.float32, kind="ExternalInput").ap()
    bcoef = nc.dram_tensor("bcoef", [W, nw * D], dt.bfloat16, kind="ExternalInput").ap()
    acoef2 = nc.dram_tensor("acoef2", [W, nw], dt.float32, kind="ExternalInput").ap()
    bcoef2 = nc.dram_tensor("bcoef2", [W, nw * D], dt.float32, kind="ExternalInput").ap()
    iota_in = nc.dram_tensor("iota", [CH, W], dt.bfloat16, kind="ExternalInput").ap()
    out = nc.dram_tensor("out", [shard, D], dt.float32, kind="ExternalOutput").ap()

    with tile.TileContext(nc) as tc:
        with (
            tc.tile_pool(name="dram", bufs=1, space="DRAM") as dram,
            tc.tile_pool(name="const", bufs=1) as const,
            tc.tile_pool(name="iop", bufs=6) as iop,
            tc.tile_pool(name="msgp", bufs=5) as msgp,
            tc.tile_pool(name="ohp", bufs=4) as ohp,
            tc.tile_pool(name="psp", bufs=8, space="PSUM") as psp,
            tc.tile_pool(name="resp", bufs=3) as resp,
        ):
            T_ins = [
                dram.tile([shard, D], dt.bfloat16, tag=f"ti{s}", name=f"T_in{s}")
                for s in range(steps)
            ]
            T_outs = [
                dram.tile([npad + 4, D], dt.bfloat16, addr_space="Shared",
                          tag=f"to{s}", name=f"T_out{s}")
                for s in range(steps)
            ]

            a_t = const.tile_from(acoef)
            b_t = const.tile_from(bcoef)
            a2_t = const.tile_from(acoef2)
            b2_t = const.tile_from(bcoef2)
            io_t = const.tile_from(iota_in)

            nc.sync.dma_start(out=T_ins[0][:, :], in_=g0[:, :])

            for step in range(steps):
                last = step == steps - 1
                T_in = T_ins[step]
                T_out = T_outs[step]
                T_in_next = T_ins[step + 1] if not last else None
                if "ag" not in ablate:
                    nc.gpsimd.collective_compute(
                        "AllGather",
                        mybir.AluOpType.bypass,
                        replica_groups=[list(range(M))],
                        ins=[T_in.opt()],
                        outs=[T_out[:npad, :].opt()],
                    )
                # 4-node-stride views of the table: view j covers src%4 == j.
                # Row r of view j = elements [64j+256r, 64j+256r+128) = the
                # 256B row starting at node 4r+j.
                flat = T_out[:, :].rearrange("n d -> (n d)")
                nr = npad // 4
                views = []
                for j in range(V):
                    vj = flat[64 * j:64 * j + nr * 256]
                    views.append(vj.rearrange("(r s) -> r s", s=256)[:, 0:128])

                ci = 0  # call index
                for s, (ks0, ks1) in enumerate(sc_ranges):
                    # gather + one-hot per call
                    sc_msgs = []  # (k0, k1, msg tile, oh tile)
                    while ci < len(calls) and calls[ci][0] == s:
                        _, vv, k0, k1 = calls[ci]
                        ncall = k1 - k0
                        idx_t = iop.tile([128, max_call * CH // 16], dt.int16,
                                         tag="idx")
                        dw_t = iop.tile([CH, max_call], dt.bfloat16, tag="dw")
                        msg_t = msgp.tile([CH, max_call, 128], dt.bfloat16,
                                          tag="msg")
                        oh_t = ohp.tile([CH, max_call, W], dt.bfloat16, tag="oh")
                        nc.sync.dma_start(
                            out=idx_t[:, :ncall * CH // 16],
                            in_=srcidx[:, k0 * CH // 16:k1 * CH // 16],
                        )
                        nc.sync.dma_start(out=dw_t[:, :ncall], in_=dstw[:, k0:k1])
                        if "gather" not in ablate:
                            step_c = subc or ncall
                            for c0 in range(0, ncall, step_c):
                                c1 = min(c0 + step_c, ncall)
                                nc.gpsimd.dma_gather(
                                    out_ap=msg_t[:, c0:c1, :],
                                    in_ap=views[vv],
                                    idxs_ap=idx_t[:, c0 * CH // 16:c1 * CH // 16],
                                    num_idxs=(c1 - c0) * CH,
                                    num_idxs_reg=(c1 - c0) * CH,
                                    elem_size=128,
                                    elem_step=256,
                                    single_packet=sp,
                                )
                        if "oh" not in ablate:
                            nc.vector.tensor_tensor(
                                out=oh_t[:, :ncall, :],
                                in0=dw_t[:, :ncall, None].to_broadcast([CH, ncall, W]),
                                in1=io_t[:, None, :].to_broadcast([CH, ncall, W]),
                                op=mybir.AluOpType.is_equal,
                            )
                        sc_msgs.append((k0, k1, msg_t, oh_t))
                        ci += 1
                    # matmul chunks + blend finished windows
                    gw_t = resp.tile([W, G, D], dt.float32, tag="gw")
                    psums = {}
                    for k0, k1, msg_t, oh_t in sc_msgs:
                        for k in range(k0, k1):
                            ww = int(wmap[k])
                            if startf[k]:
                                psums[ww] = psp.tile([W, D], dt.float32, tag="ps", name="ps")
                            if "mm" not in ablate:
                                nc.tensor.matmul(
                                    out=psums[ww][:, :],
                                    lhsT=oh_t[:, k - k0, :],
                                    rhs=msg_t[:, k - k0, 0:64],
                                    start=bool(startf[k]),
                                    stop=bool(stopf[k]),
                                )
                            if stopf[k] and "blend" not in ablate:
                                wi = ww - s * G
                                ca = (a2_t if last else a_t)[:, ww:ww + 1]
                                cb = (b2_t if last else b_t)[:, ww * D:(ww + 1) * D]
                                tmp_t = resp.tile([W, D], dt.float32, tag="tmp")
                                nc.vector.tensor_scalar(
                                    out=tmp_t[:, :], in0=psums[ww][:, :],
                                    scalar1=ca, scalar2=None,
                                    op0=mybir.AluOpType.mult,
                                )
                                nc.vector.tensor_tensor(
                                    out=gw_t[:, wi, :], in0=tmp_t[:, :], in1=cb,
                                    op=mybir.AluOpType.add,
                                )
                    # store the blended windows of this superchunk
                    if "blend" in ablate:
                        continue
                    w0 = s * G
                    nwin = min(G, nw - w0)
                    if last:
                        o32 = resp.tile([W, G, D], dt.float32, tag="o32")
                        nc.vector.tensor_copy(
                            out=o32[:, :nwin, :], in_=gw_t[:, :nwin, :]
                        )
                        nc.sync.dma_start(
                            out=out[w0 * W:(w0 + nwin) * W, :]
                                .rearrange("(a p) d -> p a d", p=W),
                            in_=o32[:, :nwin, :],
                        )
                    else:
                        g16 = resp.tile([W, G, D], dt.bfloat16, tag="g16")
                        nc.vector.tensor_copy(
                            out=g16[:, :nwin, :], in_=gw_t[:, :nwin, :]
                        )
                        nc.sync.dma_start(
                            out=T_in_next[w0 * W:(w0 + nwin) * W, :]
                                .rearrange("(a p) d -> p a d", p=W),
                            in_=g16[:, :nwin, :],
                        )
    nc.compile()
    return nc


def _make_inputs(features, src, dst, meta, M, D, alpha):
    n = features.shape[0]
    shard, npad, nw = meta["shard"], meta["npad"], meta["nw"]

    deg_out = np.bincount(src, minlength=n).astype(np.float32)
    deg_in = np.bincount(dst, minlength=n).astype(np.float32)
    ns = np.clip(deg_out, 1.0, None) ** -0.5
    nd = np.clip(deg_in, 1.0, None) ** -0.5

    ns_pad = np.ones(npad, dtype=np.float32)
    nd_pad = np.ones(npad, dtype=np.float32)
    h0_pad = np.zeros((npad, D), dtype=np.float32)
    ns_pad[:n] = ns
    nd_pad[:n] = nd
    h0_pad[:n] = features

    iota = np.tile(np.arange(W, dtype=np.float32), (CH, 1)).astype(BF16)

    in_maps = []
    for c in range(M):
        sl = slice(c * shard, (c + 1) * shard)
        a = ((1.0 - alpha) * nd_pad[sl] * ns_pad[sl]).astype(np.float32)
        b = (alpha * h0_pad[sl] * ns_pad[sl][:, None]).astype(np.float32)
        a2 = ((1.0 - alpha) * nd_pad[sl]).astype(np.float32)
        b2 = (alpha * h0_pad[sl]).astype(np.float32)
        g0 = (h0_pad[sl] * ns_pad[sl][:, None]).astype(BF16)

        def wmaj(x):  # [shard, ...] -> [W, nw * ...]
            x = x.reshape(nw, W, -1).transpose(1, 0, 2)
            return np.ascontiguousarray(x.reshape(W, -1))

        in_maps.append({
            "g0": np.ascontiguousarray(g0),
            "srcidx": meta["srcidx"][c],
            "dstw": meta["dstw_t"][c],
            "acoef": wmaj(a).astype(np.float32),
            "bcoef": wmaj(b).astype(BF16),
            "acoef2": wmaj(a2).astype(np.float32),
            "bcoef2": wmaj(b2).astype(np.float32),
            "iota": iota,
        })
    return in_maps


_NC_CACHE = {}


def build_all(features, src, dst, *, n_nodes=None, M=M_CORES, D=D_FEAT,
              steps=K_LAYERS, alpha=ALPHA, G=7, ablate=frozenset(),
              subc=None, sp=False):
    n_nodes = n_nodes or features.shape[0]
    src = np.asarray(src).astype(np.int64)
    dst = np.asarray(dst).astype(np.int64)
    meta = _preprocess(src, dst, n_nodes, M, G)
    key = (meta["nchunk"], tuple(meta["K_wv"].ravel()[:16]), steps, M, G, tuple(sorted(ablate)), subc, sp)
    if key not in _NC_CACHE:
        _NC_CACHE[key] = _build_nc(meta, M, D, steps, frozenset(ablate), subc, sp)
    nc = _NC_CACHE[key]
    in_maps = _make_inputs(np.asarray(features, np.float32), src, dst, meta, M, D, alpha)
    return nc, in_maps, meta


def kernel(features, src, dst, *, trace=False, **kw):
    from concourse.bass_utils import run_bass_kernel_spmd

    features = np.asarray(features)
    nc, in_maps, meta = build_all(features, src, dst, **kw)
    res = run_bass_kernel_spmd(nc, in_maps, core_ids=list(range(M_CORES)), trace=trace)
    n = features.shape[0]
    h = np.concatenate([res.results[c]["out"] for c in range(M_CORES)], axis=0)
    out = np.ascontiguousarray(h[:n]).astype(np.float32)
    if trace:
        kernel.last_results = res
    return out
